# revision 20
# baseline (speedup 1.0000x reference)
"""DCPNet rigid-alignment head on 8 Trainium2 NeuronCores.

Data-parallel over batch: B=16 samples -> 2 per core. Per sample the device
computes the O(N^2 D) part:
  pd[m,n]  = ||se_n||^2 - 2 te_m . se_n + ||te_m||^2  (bf16 PE matmul over 4
             K-chunks + one bf16 augmented K=2 matmul adding the centered
             -0.5*xx+256 / -0.5*yy+256 rows, host-precomputed)
  pdc      = -2*psum  (DVE drain of PSUM -> SBUF as bf16; equals pd-1024,
             centered so bf16 quantization stays ~0.25 abs on pd ~1024+-300)
  d        = Sqrt(pdc + 1024)   (ACT, sqrt table, fp32 out)
  E        = Exp(-d)            (ACT, exp table, bf16 out)
  C[j,n]   = sum_m E[m,n] * [tgt;1][j,m]   (bf16 PE matmul -> [4, N])
The host does the O(N) tail: corr = C[0:3]/C[3], cross-covariance H with
srcs, 3x3 SVD -> R, t, euler angles.

Schedule notes:
- ACT is the hard floor: 1 elem/cycle/lane @1.2GHz, two passes over both
  N*N score matrices = ~32us. Everything else hides behind it or the PE.
- ACT table phases (4 loads, enforced via tile_wait_until sim-time gates
  because the Tile scheduler is readiness-ordered, not program-ordered):
  sqrt0 chases A0's drains | exp0 overlaps A1 | sqrt1 chases A1 | exp1 tail.
- All inputs ship as bf16; embeddings are host-permuted to partition-major
  [128, K*N] so every DMA descriptor is a contiguous 4KB row. tgts ships as
  [tgts;1] [4,N] and is transposed on-chip via a K=4 matmul with an
  identity (a strided "n d" DMA would emit ~13k 2-byte descriptors).
- No DVE memsets on the critical path: the ones rows ride in from the host.
"""

import sys

if "/opt/trn_rl_repo" not in sys.path:
    sys.path.insert(0, "/opt/trn_rl_repo")

import numpy as np

_B, _N, _D = 16, 1024, 512
_NCORES = 8
_SPC = _B // _NCORES  # samples per core

_state = {}


def _patch_act_tables():
    """Constrain the ACT table sets so the load inserter emits exactly 4 loads:
    Sqrt lives only in sqrt_and_others, Exp only in exp_and_others."""
    from concourse import bacc, hw_specs, mybir

    if getattr(bacc, "_dcp_act_patch", False):
        return
    orig = hw_specs.get_activation_tables

    def patched(module_arch):
        tables = dict(orig(module_arch))
        AF = mybir.ActivationFunctionType
        for name, funcs in tables.items():
            if name != "sqrt_and_others":
                funcs.difference_update({AF.Sqrt, AF.Square})
            if name != "natural_log_exp_and_others":
                funcs.difference_update({AF.Exp, AF.Ln})
        return tables

    bacc.get_activation_tables = patched
    hw_specs.get_activation_tables = patched
    bacc._dcp_act_patch = True


def _enable_ldw_opt():
    """Flip walrus's --enable-ldw-opt to true so matmuls sharing a stationary
    operand keep a single LDWEIGHTS (works now that the kernel has no
    is_transpose matmuls, which that pass can't handle)."""
    from concourse import bass_utils

    if getattr(bass_utils, "_dcp_ldw_patch", False):
        return
    orig = bass_utils.run_command

    def patched(cmd, *a, **kw):
        if isinstance(cmd, list):
            cmd = [
                "--enable-ldw-opt=true" if c == "--enable-ldw-opt=false" else c
                for c in cmd
            ]
        return orig(cmd, *a, **kw)

    bass_utils.run_command = patched
    bass_utils._dcp_ldw_patch = True


def _build():
    if "nc" in _state:
        return _state["nc"]

    from contextlib import ExitStack

    import concourse.tile as tile
    from concourse import bacc, mybir
    from concourse.masks import make_identity

    _patch_act_tables()

    fp32 = mybir.dt.float32
    bf16 = mybir.dt.bfloat16
    AF = mybir.ActivationFunctionType
    ALU = mybir.AluOpType

    KC = _D // 128  # 4 contraction chunks
    MC = _N // 128  # 8 partition chunks of the score matrix
    NH = _N // 512  # 2 free-dim halves (PSUM bank = 512 fp32)

    nc = bacc.Bacc()
    tgts4 = nc.declare_dram_parameter("tgts4", [_SPC, 4, _N], bf16, isOutput=False)
    semb = nc.declare_dram_parameter(
        "srcs_emb", [_SPC, 128, KC * _N], bf16, isOutput=False
    )
    temb = nc.declare_dram_parameter(
        "tgts_emb", [_SPC, 128, KC * _N], bf16, isOutput=False
    )
    augl = nc.declare_dram_parameter("augl", [_SPC, 2, _N], bf16, isOutput=False)
    augr = nc.declare_dram_parameter("augr", [_SPC, 2, _N], bf16, isOutput=False)
    c_out = nc.declare_dram_parameter("c_out", [_SPC, 4, _N], fp32, isOutput=True)

    with ExitStack() as ctx:
        tc = ctx.enter_context(tile.TileContext(nc))
        singles = ctx.enter_context(tc.tile_pool(name="singles", bufs=1))
        emb = ctx.enter_context(tc.tile_pool(name="emb", bufs=2))
        pdp = ctx.enter_context(tc.tile_pool(name="pdp", bufs=2))
        ddp = ctx.enter_context(tc.tile_pool(name="ddp", bufs=2))
        eep = ctx.enter_context(tc.tile_pool(name="eep", bufs=4))
        ee0 = ctx.enter_context(tc.tile_pool(name="ee0", bufs=8))
        small = ctx.enter_context(tc.tile_pool(name="small", bufs=2))
        # PSUM (8 banks): g2 2 banks x 2 bufs, c2 2 banks x 1, small 1 bank x 2
        psg = ctx.enter_context(tc.tile_pool(name="psg", bufs=2, space="PSUM"))
        psc = ctx.enter_context(tc.tile_pool(name="psc", bufs=1, space="PSUM"))
        pss = ctx.enter_context(tc.tile_pool(name="pss", bufs=2, space="PSUM"))

        ident4 = singles.tile([4, 4], bf16)
        make_identity(nc, ident4)
        b1024 = singles.tile([128, 1], fp32)
        nc.vector.memset(b1024, 1024.0)
        junk4 = singles.tile([4, 512], bf16)
        nc.vector.memset(junk4, 0.0)

        def emit_prewarm(n, tag):
            """Dummy matmuls that spin the PE so its DVFS ramps to max clock
            before (or across) a real-work gap."""
            wp = pss.tile([4, 512], fp32, tag="ps1", name=f"wp_{tag}")
            for _ in range(n):
                nc.tensor.matmul(wp, junk4[:, 0:4], junk4, start=True, stop=True)
            wsb = singles.tile([4, 1], fp32, tag=f"wsb{tag}", name=f"wsb_{tag}")
            nc.vector.tensor_copy(wsb, wp[:, 0:1])

        se_t, te_t, st4, ptT, aug_lhsT, aug_rhs, pd_sb, d_sb, c2 = (
            [None] * _SPC for _ in range(9)
        )

        def emit_loads(s):
            """DMA for one sample: the sample's se/te halves are split across
            both HW queues so the first sample lands as early as possible; the
            first sample's aug rows lead the sync queue (needed by m0's aug
            matmul), everything else rides behind."""
            aug_lhsT[s] = small.tile([2, _N], bf16, tag="auglhs", name=f"al{s}")
            aug_rhs[s] = small.tile([2, _N], bf16, tag="augrhs", name=f"ar{s}")
            st4[s] = small.tile([4, _N], bf16, tag="st4", name=f"st4_{s}")
            se_t[s] = emb.tile([128, KC, _N], bf16, tag="se", name=f"se{s}")
            te_t[s] = emb.tile([128, KC, _N], bf16, tag="te", name=f"te{s}")
            if s == 0:
                nc.sync.dma_start(out=aug_lhsT[s], in_=augl[s])
                nc.sync.dma_start(out=aug_rhs[s], in_=augr[s])
                nc.sync.dma_start(out=st4[s], in_=tgts4[s])
            se_flat = se_t[s].rearrange("p k n -> p (k n)")
            te_flat = te_t[s].rearrange("p k n -> p (k n)")
            HF = 2 * _N
            nc.sync.dma_start(out=se_flat[:, 0:HF], in_=semb[s][:, 0:HF])
            nc.scalar.dma_start(out=te_flat[:, 0:HF], in_=temb[s][:, 0:HF])
            nc.scalar.dma_start(out=se_flat[:, HF:], in_=semb[s][:, HF:])
            nc.sync.dma_start(out=te_flat[:, HF:], in_=temb[s][:, HF:])
            if s == 1:
                nc.sync.dma_start(out=st4[s], in_=tgts4[s])
                nc.sync.dma_start(out=aug_lhsT[s], in_=augl[s])
                nc.sync.dma_start(out=aug_rhs[s], in_=augr[s])

            pd_sb[s] = pdp.tile([128, MC, _N], bf16, tag="pd", name=f"pd{s}")
            d_sb[s] = ddp.tile([128, MC, _N], fp32, tag="dd", name=f"dd{s}")

        def emit_ptT(s):
            """Transpose [tgts;1] (4 x N) into [128, q, 4] chunks via a K=4
            matmul against the 4x4 identity."""
            ptT[s] = small.tile([128, MC, 4], bf16, tag="ptT", name=f"ptT{s}")
            for q in range(MC):
                ps4 = pss.tile([128, 4], fp32, tag="ps1", name=f"pt{s}{q}")
                nc.tensor.matmul(
                    ps4, st4[s][:, q * 128 : (q + 1) * 128], ident4,
                    start=True, stop=True,
                )
                nc.vector.tensor_copy(ptT[s][:, q, :], ps4)

        def emit_mtile(s, m, with_sqrt):
            """One m-tile of the score matrix: PE matmuls -> PSUM, DVE drain to
            centered-bf16 SBUF, optionally the ACT sqrt right away."""
            msl = slice(m * 128, (m + 1) * 128)
            g2 = psg.tile([128, NH, 512], fp32, tag="g2", name=f"g2_{s}{m}")
            for k in range(KC):
                for nh in range(NH):
                    nc.tensor.matmul(
                        g2[:, nh, :],
                        te_t[s][:, k, msl],
                        se_t[s][:, k, nh * 512 : (nh + 1) * 512],
                        start=(k == 0),
                        stop=False,
                    )
            for nh in range(NH):
                nc.tensor.matmul(
                    g2[:, nh, :],
                    aug_lhsT[s][:, msl],
                    aug_rhs[s][:, nh * 512 : (nh + 1) * 512],
                    start=False,
                    stop=True,
                )
            # psum = inner - 0.5xx - 0.5yy + 512  ->  pdc = -2*psum = pd - 1024
            nc.vector.tensor_scalar(
                out=pd_sb[s][:, m, :],
                in0=g2.rearrange("p a b -> p (a b)"),
                scalar1=-2.0,
                scalar2=None,
                op0=ALU.mult,
            )
            if with_sqrt:
                emit_sqrt(s, m, m + 1)

        def emit_sqrt(s, m0, m1):
            nc.scalar.activation(
                out=d_sb[s][:, m0:m1, :],
                in_=pd_sb[s][:, m0:m1, :],
                func=AF.Sqrt,
                bias=b1024[:, 0:1],
            )

        eg0 = [None] * MC

        def emit_lnexp(s, m):
            """3-pass d/E for one tile on the single ln+exp table set:
            d = exp(0.5*ln(pd)), E = exp(-d). ACT only -- the E-matmuls are
            emitted later so the PE's in-order stream never waits on the
            ACT backlog mid A-phase."""
            L = d_sb[s][:, m, :]
            nc.scalar.activation(
                out=L, in_=pd_sb[s][:, m, :], func=AF.Ln, bias=b1024[:, 0:1]
            )
            nc.scalar.activation(out=L, in_=L, func=AF.Exp, scale=0.5)
            eg0[m] = ee0.tile([128, _N], bf16, tag="eg0", name=f"eg0_{m}")
            nc.scalar.activation(out=eg0[m], in_=L, func=AF.Exp, scale=-1.0)

        def emit_e0(s, m):
            for nh in range(NH):
                nc.tensor.matmul(
                    c2[s][:, nh, :],
                    ptT[s][:, m, :],
                    eg0[m][:, nh * 512 : (nh + 1) * 512],
                    start=(m == 0),
                    stop=(m == MC - 1),
                )

        def emit_exp_e(s, m0, m1):
            """Exp over m-tiles [m0, m1) + the E-matmul pairs into c2."""
            eg = eep.tile([128, 4, _N], bf16, tag="eg", name=f"eg{s}{m0}")
            nc.scalar.activation(
                out=eg[:, 0 : m1 - m0, :], in_=d_sb[s][:, m0:m1, :], func=AF.Exp,
                scale=-1.0,
            )
            for j in range(m1 - m0):
                m = m0 + j
                for nh in range(NH):
                    nc.tensor.matmul(
                        c2[s][:, nh, :],
                        ptT[s][:, m, :],
                        eg[:, j, nh * 512 : (nh + 1) * 512],
                        start=(m == 0),
                        stop=(m == MC - 1),
                    )

        def emit_ctail(s):
            """Ship the unnormalized correspondence matrix C to the host."""
            c_sb = small.tile([4, NH, 512], fp32, tag="csb", name=f"csb{s}")
            nc.vector.tensor_copy(c_sb, c2[s])
            nc.sync.dma_start(
                out=c_out[s], in_=c_sb.rearrange("p a b -> p (a b)")
            )

        # ---- schedule ----
        c2[0] = psc.tile([4, NH, 512], fp32, tag="c2", name="c2_0")

        with tc.high_priority():
            emit_loads(0)
            emit_loads(1)
        emit_prewarm(12, "a")
        emit_ptT(0)
        # sample 0: 3-pass ln/exp/exp rides the ln+exp table set per tile --
        # the extra pass hides in the drain-supply gaps, and no table switch
        # is needed until sample 1's sqrt phase
        for m in range(MC):
            emit_mtile(0, m, with_sqrt=False)
            emit_lnexp(0, m)
        emit_ptT(1)
        for m in range(MC):
            emit_mtile(1, m, with_sqrt=False)
        with tc.tile_wait_until(0.5):
            for m in range(MC):
                emit_e0(0, m)
        emit_ctail(0)
        # phase gates: sim-time floors order the single ACT engine's stream
        # (lnexp0 | sqrt1 | exp1) so the table loads don't thrash
        with tc.tile_wait_until(2):
            for m0, m1 in ((0, 2), (2, 4), (4, 6), (6, 7), (7, 8)):
                emit_sqrt(1, m0, m1)
        c2[1] = psc.tile([4, NH, 512], fp32, tag="c2", name="c2_1")
        with tc.tile_wait_until(2.5):
            emit_prewarm(14, "b")
        with tc.tile_wait_until(3):
            for m0, m1 in ((0, 4), (4, 6), (6, 7), (7, 8)):
                emit_exp_e(1, m0, m1)
        emit_ctail(1)

    nc.finalize()
    _state["nc"] = nc
    return nc


def _postprocess(c_all, srcs):
    """c_all: [B, 4, N] unnormalized correspondence sums; srcs: [B, 3, N].
    Host tail: normalize, cross-covariance, 3x3 SVD -> [B, 6]."""
    c = c_all.astype(np.float64)
    s = np.asarray(srcs, dtype=np.float64)
    corr = c[:, 0:3, :] / c[:, 3:4, :]
    sm = s.mean(axis=2, keepdims=True)
    cm = corr.mean(axis=2, keepdims=True)
    H = np.einsum("bin,bjn->bij", s - sm, corr - cm)
    u, _, vh = np.linalg.svd(H)
    v = np.swapaxes(vh, -1, -2)
    r = v @ np.swapaxes(u, -1, -2)
    det = np.linalg.det(r)
    flip = np.where(det[:, None] < 0, np.array([1.0, 1.0, -1.0]), 1.0)
    v = v * flip[:, None, :]
    R = v @ np.swapaxes(u, -1, -2)
    t = -np.einsum("bij,bjk->bik", R, sm)[:, :, 0] + cm[:, :, 0]
    cy = np.sqrt(R[:, 2, 2] ** 2 + R[:, 1, 2] ** 2)
    ax = np.arctan2(-R[:, 1, 2], R[:, 2, 2])
    ay = np.arctan2(R[:, 0, 2], cy)
    az = np.arctan2(-R[:, 0, 1], R[:, 0, 0])
    return np.concatenate([np.stack([ax, ay, az], 1), t], axis=1).astype(np.float32)


def kernel(srcs, tgts, srcs_emb, tgts_emb, **run_kwargs):
    import ml_dtypes

    from concourse.bass_utils import run_bass_kernel_spmd

    nc = _build()
    bf = ml_dtypes.bfloat16

    tgts4 = np.ones((_B, 4, _N), dtype=np.float32)
    tgts4[:, 0:3, :] = np.asarray(tgts, dtype=np.float32)
    tgts4 = tgts4.astype(bf)

    def permute_emb(e):
        # [B, 512, 1024] -> [B, 128, 4*1024] with row p = chunks k at d=k*128+p
        e = np.asarray(e, dtype=np.float32).reshape(_B, 4, 128, _N)
        return np.ascontiguousarray(e.transpose(0, 2, 1, 3)).reshape(
            _B, 128, 4 * _N
        ).astype(bf)

    semb_b = permute_emb(srcs_emb)
    temb_b = permute_emb(tgts_emb)

    se32 = np.asarray(srcs_emb, dtype=np.float32)
    te32 = np.asarray(tgts_emb, dtype=np.float32)
    # centered aug rows: psum gets inner - 0.5xx - 0.5yy + 512
    augl_b = np.ones((_B, 2, _N), dtype=np.float32)
    augl_b[:, 0, :] = 256.0 - 0.5 * (te32 * te32).sum(axis=1)
    augr_b = np.ones((_B, 2, _N), dtype=np.float32)
    augr_b[:, 1, :] = 256.0 - 0.5 * (se32 * se32).sum(axis=1)
    augl_b = augl_b.astype(bf)
    augr_b = augr_b.astype(bf)

    in_maps = []
    for c in range(_NCORES):
        sl = slice(c * _SPC, (c + 1) * _SPC)
        in_maps.append(
            {
                "tgts4": np.ascontiguousarray(tgts4[sl]),
                "srcs_emb": np.ascontiguousarray(semb_b[sl]),
                "tgts_emb": np.ascontiguousarray(temb_b[sl]),
                "augl": np.ascontiguousarray(augl_b[sl]),
                "augr": np.ascontiguousarray(augr_b[sl]),
            }
        )
    res = run_bass_kernel_spmd(nc, in_maps, list(range(_NCORES)), **run_kwargs)
    c_all = np.concatenate(
        [np.asarray(res.results[c]["c_out"]) for c in range(_NCORES)], axis=0
    )
    out = _postprocess(c_all, srcs)
    if run_kwargs:
        _state["last_results"] = res
    return out
